# revision 41
# baseline (speedup 1.0000x reference)
"""Binarized 3x3 conv (NCHW, VALID, stride 1) on 8 Trainium2 NeuronCores.

Reference: out = conv2d(X, sign(W)) with X [32,256,56,56] f32, W [256,256,3,3]
f32 (OIHW), out [32,256,54,54].

Sharding (per the data-parallel hint): each of the 8 cores gets 4 images of
the batch; the (tiny) weight is replicated.  No collectives.  The host only
re-lays-out W to [kh*kw, ci, co] (pure transpose, no arithmetic); sign() runs
on device.

kernel() uses build_conv_bass_wino(w_fp8=True): 1D Winograd F(2,3) along
width with bf16 comps and fp8e4 stationary weights.  The transformed weight
values {+-1/2, +-1, +-3/2} are exact in fp8e4, so numerics are bit-identical
to the bf16-weight kernel (rel err 2.19e-3 vs the f32 reference, from input
bf16 rounding only); the narrower weight rows shave a few cycles off each
InstLdweights.

Measured per-matmul cost on this hw path (For_i wall-slope fits):
[K=128,M=128,N=486] bf16 ~ 637 cyc = 486 stream + ~128 serial weight-load +
~23 fixed — InstLdweights does NOT overlap matmul streaming, so the kernel
is PE-bound at 576 x 637 cyc / 4 images ~ 153us/core, which matches the
measured E.  Explored and rejected (all slower on hw):
  - fp8 DoubleRow hi/lo split (2x stream on paper): accumulating (start=
    False) DR matmuls run at 1 row/cycle, so chains see no stream gain and
    pay a 2x weight load (measured ~204us kernel).
  - float32r: birsim deadlocks on back-to-back f32r matmuls inside For_i.
  - PSUM tiles >512 f32 (to amortize the weight-load over larger N):
    matmul out free size is ISA-capped at 512 (s3d3_mm_num_elements).
  - F(4,3) along W (25% fewer PE stream cycles): correct (rel 8.9e-3) but
    every structural variant measured 330-480us; even a matmul-only
    ablation of that shape runs ~2.5x slower per matmul than this kernel's
    steady state (unresolved; suspected PE p-state resets from pipeline
    gaps: TRN2 PE runs at 1.2GHz after any idle gap and needs 3us of
    continuous execution to reach 2.4GHz).

Other builders (build_conv_bass, build_conv_fp8wino, build_conv_f43) are
kept for reference; kernel() does not use them.
"""

import numpy as np

_N, _C, _H, _W = 32, 256, 56, 56
_CO, _KH, _KW = 256, 3, 3
_HO, _WO = 54, 54
_NCORES = 8
_NPC = _N // _NCORES  # images per core

_R = 9             # output rows per PSUM group
_G = _HO // _R     # 6 row groups
_NF = _R * _W      # 504 = matmul free size
_PAD = 8           # bf16 image pad so the last rhs slice stays in bounds


def build_conv_bass(
    npc=_NPC,
    reps=1,
    free2d=True,
    w_on_act=True,
    cast_chunks=6,
    hw_loop=0,
    wswap=False,
    xb_bufs=4,
    ob_bufs=3,
):
    import contextlib

    import concourse.mybir as mybir
    import concourse.tile as tile
    from concourse import bacc

    fp32 = mybir.dt.float32
    bf16 = mybir.dt.bfloat16

    nc = bacc.Bacc("TRN2", target_bir_lowering=False, debug=False)

    x_in = nc.dram_tensor("x", [npc, _C, _H, _W], fp32, kind="ExternalInput")
    w_in = nc.dram_tensor("w", [_KH * _KW, _C, _CO], fp32, kind="ExternalInput")
    out = nc.dram_tensor("out", [npc, _CO, _HO, _WO], fp32, kind="ExternalOutput")

    n_ci = _C // 128   # 2
    n_co = _CO // 128  # 2
    nk = _KH * _KW     # 9

    with tile.TileContext(nc) as tc:
        with (
            tc.tile_pool(name="wstage", bufs=2) as wstage_pool,
            tc.tile_pool(name="wb", bufs=n_ci) as wb_pool,
            tc.tile_pool(name="xf", bufs=2) as xf_pool,
            tc.tile_pool(name="xb", bufs=xb_bufs) as xb_pool,
            tc.tile_pool(name="ob", bufs=ob_bufs) as ob_pool,
            tc.tile_pool(name="ps", bufs=8, space="PSUM") as ps_pool,
        ):
            # ---- weight prep: one DMA + one binarize per ci tile.
            # wb[ci_t][:, khw, co] = 0.5*sign(W[co, ci, khw]) in bf16 (exact)
            wb = {}
            w_dma_eng = nc.scalar if w_on_act else nc.sync
            for ci_t in range(n_ci):
                stage = wstage_pool.tile([128, nk, _CO], fp32)
                w_dma_eng.dma_start(
                    stage[:],
                    w_in[:, ci_t * 128 : (ci_t + 1) * 128, :].rearrange(
                        "k c o -> c k o"
                    ),
                )
                wt = wb_pool.tile([128, nk, _CO], bf16)
                # (w >= 0) -> {1,0}; minus 0.5 -> {+0.5,-0.5} == sign(w)/2
                nc.vector.tensor_scalar(
                    wt[:], stage[:], 0.0, 0.5,
                    mybir.AluOpType.is_ge, mybir.AluOpType.subtract,
                )
                wb[ci_t] = wt

            # ---- main loop over images
            # hw_loop>0: wrap the whole image loop in a device-side For_i
            # (loop var unused; used only to scale exec time for benchmarking)
            loop_cm = (
                tc.For_i(
                    0,
                    hw_loop,
                    1,
                    hint_engines=(
                        mybir.EngineType.PE,
                        mybir.EngineType.Activation,
                        mybir.EngineType.DVE,
                        mybir.EngineType.SP,
                    ),
                )
                if hw_loop > 0
                else contextlib.nullcontext()
            )
            with loop_cm:
                for rep in range(reps):
                    for n in range(npc):
                        xb = {}
                        for ci_t in range(n_ci):
                            xf = xf_pool.tile([128, _H * _W], fp32)
                            nc.sync.dma_start(
                                xf[:], x_in[n, ci_t * 128 : (ci_t + 1) * 128, :, :]
                            )
                            xt = xb_pool.tile([128, _H * _W + _PAD], bf16)
                            nc.vector.memset(xt[:, _H * _W :], 0.0)
                            # chunked cast so the first matmuls start sooner
                            hw = _H * _W
                            step = -(-hw // cast_chunks)
                            for s in range(0, hw, step):
                                e = min(s + step, hw)
                                nc.vector.tensor_copy(xt[:, s:e], xf[:, s:e])
                            xb[ci_t] = xt
    
                        for co_t in range(n_co):
                            pshape = [128, _R, _WO] if free2d else [128, _R, _W]
                            psts = [
                                ps_pool.tile(pshape, fp32, name="pst", tag="pst")
                                for _ in range(_G)
                            ]
                            taps = [
                                (ci_t, kh, kw)
                                for ci_t in range(n_ci)
                                for kh in range(_KH)
                                for kw in range(_KW)
                            ]
                            # weight-stationary (w outer, g inner) unless wswap
                            mm_iter = (
                                [(t, g) for g in range(_G) for t in taps]
                                if wswap
                                else [(t, g) for t in taps for g in range(_G)]
                            )
                            for (ci_t, kh, kw), g in mm_iter:
                                w_ap = wb[ci_t][
                                    :, kh * _KW + kw,
                                    co_t * 128 : (co_t + 1) * 128,
                                ]
                                first = ci_t == 0 and kh == 0 and kw == 0
                                last = (
                                    ci_t == n_ci - 1
                                    and kh == _KH - 1
                                    and kw == _KW - 1
                                )
                                base = (g * _R + kh) * _W + kw
                                if free2d:
                                    rhs = xb[ci_t][
                                        :, base : base + _NF
                                    ].rearrange("p (r w) -> p r w", r=_R)[
                                        :, :, 0:_WO
                                    ]
                                else:
                                    rhs = xb[ci_t][:, base : base + _NF]
                                nc.tensor.matmul(
                                    psts[g][:, :, :],
                                    w_ap,
                                    rhs,
                                    start=first,
                                    stop=last,
                                )
                            ob = ob_pool.tile([128, _HO, _WO], fp32)
                            for g in range(_G):
                                # x2 undoes the +-0.5 weight encoding (exact)
                                nc.scalar.mul(
                                    ob[:, g * _R : (g + 1) * _R, :],
                                    psts[g][:, :, 0:_WO] if not free2d else psts[g][:],
                                    2.0,
                                )
                            nc.scalar.dma_start(
                                out[n, co_t * 128 : (co_t + 1) * 128, :, :], ob[:]
                            )

    nc.compile()
    return nc


def build_conv_bass_wino(
    npc=_NPC,
    reps=1,
    hw_loop=0,
    rg=18,
    xw_bufs=4,
    ob_bufs=2,
    dt_bufs=8,
    tr_chunks=3,
    w_fp8=True,
    wave2=False,
    xf_bufs=2,
):
    """1D Winograd F(2,3) along width: 1.5x fewer PE cycles than direct.

    For output pair (y0,y1) at width 2j with taps (w0,w1,w2) per (ci,kh):
      c1=d0-d2, c2=d1+d2, c3=d2-d1, c4=d1-d3   (dt = x[2j+t])
      m1=c1*w0, m2=c2*(w0+w1+w2)/2, m3=c3*(w0-w1+w2)/2, m4=c4*w2
      y0 = m1+m2+m3,  y1 = m2-m3-m4
    Signs are binarized first, so transformed weights are in
    {+-1, +-1/2, +-3/2} — exact in bf16.  Input comps are computed on DVE
    straight from the f32 image (cast fused).  The four m accumulators are
    one PSUM quad per row-group; detransform is 4 DVE adds per quad.
    """
    import contextlib

    import concourse.mybir as mybir
    import concourse.tile as tile
    from concourse import bacc

    fp32 = mybir.dt.float32
    bf16 = mybir.dt.bfloat16
    wdt = mybir.dt.float8e4 if w_fp8 else bf16

    nc = bacc.Bacc("TRN2", target_bir_lowering=False, debug=False)

    x_in = nc.dram_tensor("x", [npc, _C, _H, _W], fp32, kind="ExternalInput")
    w_in = nc.dram_tensor("w", [_KH * _KW, _C, _CO], fp32, kind="ExternalInput")
    out = nc.dram_tensor("out", [npc, _CO, _HO, _WO], fp32, kind="ExternalOutput")

    n_ci = _C // 128   # 2
    n_co = _CO // 128  # 2
    nk = _KH * _KW     # 9
    WP = _WO // 2      # 27 output pairs
    G = _HO // rg      # row groups (rg=18 -> 3)
    assert _HO % rg == 0

    with tile.TileContext(nc) as tc:
        with (
            tc.tile_pool(name="wstage", bufs=1) as wstage_pool,
            tc.tile_pool(name="wsign", bufs=1) as wsign_pool,
            tc.tile_pool(name="wtmp", bufs=2) as wtmp_pool,
            tc.tile_pool(name="wg", bufs=n_ci) as wg_pool,
            tc.tile_pool(name="xf", bufs=xf_bufs) as xf_pool,
            tc.tile_pool(name="xw", bufs=xw_bufs) as xw_pool,
            tc.tile_pool(name="dt", bufs=dt_bufs) as dt_pool,
            tc.tile_pool(name="ob", bufs=ob_bufs) as ob_pool,
            tc.tile_pool(name="ps", bufs=8, space="PSUM") as ps_pool,
        ):
            # ---- weight prep: binarize then 1D-transform along kw.
            # wg[ci_t][:, kh*4 + c, co]: c=0 -> s0, c=1 -> (s0+s1+s2)/2,
            # c=2 -> (s0-s1+s2)/2, c=3 -> s2   (s = sign(w))
            wg = {}
            for ci_t in range(n_ci):
                stage = wstage_pool.tile([128, nk, _CO], fp32)
                nc.scalar.dma_start(
                    stage[:],
                    w_in[:, ci_t * 128 : (ci_t + 1) * 128, :].rearrange(
                        "k c o -> c k o"
                    ),
                )
                sg = wsign_pool.tile([128, nk, _CO], fp32)
                nc.vector.tensor_scalar(
                    sg[:], stage[:], 0.0, 0.5,
                    mybir.AluOpType.is_ge, mybir.AluOpType.subtract,
                )  # +-0.5 = sign/2
                wt = wg_pool.tile([128, _KH * 4, _CO], wdt)
                for kh in range(_KH):
                    s0 = sg[:, kh * _KW + 0, :]
                    s1 = sg[:, kh * _KW + 1, :]
                    s2 = sg[:, kh * _KW + 2, :]
                    # c=0: s0 (x2 undoes the half encoding)
                    nc.scalar.mul(wt[:, kh * 4 + 0, :], s0, 2.0)
                    # c=3: s2
                    nc.scalar.mul(wt[:, kh * 4 + 3, :], s2, 2.0)
                    t02 = wtmp_pool.tile([128, _CO], fp32, name="t02", tag="t02")
                    nc.vector.tensor_add(t02[:], s0, s2)
                    tp = wtmp_pool.tile([128, _CO], fp32, name="tp", tag="tp")
                    nc.vector.tensor_add(tp[:], t02[:], s1)
                    nc.scalar.copy(wt[:, kh * 4 + 1, :], tp[:])  # (s0+s1+s2)/2
                    tm = wtmp_pool.tile([128, _CO], fp32, name="tm", tag="tm")
                    nc.vector.tensor_sub(tm[:], t02[:], s1)
                    nc.scalar.copy(wt[:, kh * 4 + 2, :], tm[:])  # (s0-s1+s2)/2
                wg[ci_t] = wt

            loop_cm = (
                tc.For_i(
                    0,
                    hw_loop,
                    1,
                    hint_engines=(
                        mybir.EngineType.PE,
                        mybir.EngineType.Activation,
                        mybir.EngineType.DVE,
                        mybir.EngineType.SP,
                    ),
                )
                if hw_loop > 0
                else contextlib.nullcontext()
            )
            with loop_cm:
                for rep in range(reps):
                    for n in range(npc):
                        # input comps straight from f32 (cast fused into sub/add)
                        xw = {}
                        for ci_t in range(n_ci):
                            xf = xf_pool.tile([128, _H * _W], fp32)
                            nc.sync.dma_start(
                                xf[:], x_in[n, ci_t * 128 : (ci_t + 1) * 128, :, :]
                            )
                            v = xf[:].rearrange(
                                "p (h wp t) -> p h wp t", wp=_W // 2, t=2
                            )
                            d0 = v[:, :, 0:WP, 0]
                            d1 = v[:, :, 0:WP, 1]
                            d2 = v[:, :, 1 : WP + 1, 0]
                            d3 = v[:, :, 1 : WP + 1, 1]
                            xc = xw_pool.tile(
                                [128, 4, _H, WP], bf16, name="xc", tag="xc"
                            )
                            # chunk over rows so group-0 matmuls start before
                            # the whole image is transformed
                            hstep = -(-_H // tr_chunks)
                            for h0 in range(0, _H, hstep):
                                h1 = min(h0 + hstep, _H)
                                r = slice(h0, h1)
                                nc.vector.tensor_sub(
                                    xc[:, 0, r], d0[:, r], d2[:, r]
                                )  # c1
                                nc.vector.tensor_add(
                                    xc[:, 1, r], d1[:, r], d2[:, r]
                                )  # c2
                                nc.vector.tensor_sub(
                                    xc[:, 2, r], d2[:, r], d1[:, r]
                                )  # c3
                                nc.vector.tensor_sub(
                                    xc[:, 3, r], d1[:, r], d3[:, r]
                                )  # c4
                            xw[ci_t] = xc

                        for co_t in range(n_co):
                            ob = ob_pool.tile(
                                [128, _HO, _WO], fp32, name="ob", tag="ob"
                            )
                            obv = ob[:].rearrange("p h (wp t) -> p h wp t", t=2)
                            waves = ((0, 1), (2,)) if wave2 else ((0,), (1,), (2,))
                            for wave in waves:
                                mqs = {}
                                for g in wave:
                                    mqs[g] = [
                                        ps_pool.tile(
                                            [128, rg, WP], fp32,
                                            name="mq", tag="mq",
                                        )
                                        for _ in range(4)
                                    ]
                                for c in range(4):
                                    for ci_t in range(n_ci):
                                        for kh in range(_KH):
                                            w_ap = wg[ci_t][
                                                :, kh * 4 + c,
                                                co_t * 128 : (co_t + 1) * 128,
                                            ]
                                            st = ci_t == 0 and kh == 0
                                            sp = (
                                                ci_t == n_ci - 1
                                                and kh == _KH - 1
                                            )
                                            for g in wave:
                                                rhs = xw[ci_t][
                                                    :, c,
                                                    g * rg + kh
                                                    : g * rg + kh + rg,
                                                    :,
                                                ]
                                                nc.tensor.matmul(
                                                    mqs[g][c][:, :, :],
                                                    w_ap,
                                                    rhs,
                                                    start=st,
                                                    stop=sp,
                                                    skip_group_check=wave2,
                                                )
                                for g in wave:
                                    mq = mqs[g]
                                    # detransform: y0=m1+m2+m3, y1=m2-m3-m4.
                                    # DVE may read only ONE psum operand per
                                    # op; ACT stages m2,m3 into SBUF first.
                                    rows = slice(g * rg, (g + 1) * rg)
                                    s2 = dt_pool.tile([128, rg, WP], fp32, name="s2", tag="s2")
                                    nc.scalar.copy(s2[:], mq[1][:])
                                    s3 = dt_pool.tile([128, rg, WP], fp32, name="s3", tag="s3")
                                    nc.scalar.copy(s3[:], mq[2][:])
                                    t0 = dt_pool.tile([128, rg, WP], fp32, name="t0", tag="t0")
                                    nc.vector.tensor_add(t0[:], mq[0][:], s2[:])
                                    nc.vector.tensor_add(
                                        obv[:, rows, :, 0], t0[:], s3[:]
                                    )
                                    t1 = dt_pool.tile([128, rg, WP], fp32, name="t1", tag="t1")
                                    nc.vector.tensor_sub(t1[:], s2[:], s3[:])
                                    nc.vector.tensor_sub(
                                        obv[:, rows, :, 1], t1[:], mq[3][:]
                                    )
                                    if g == G - 1:
                                        nc.scalar.dma_start(
                                            out[
                                                n,
                                                co_t * 128 : (co_t + 1) * 128,
                                                :,
                                                :,
                                            ],
                                            ob[:],
                                        )

    nc.compile()
    return nc


def build_conv_fp8wino(
    npc=_NPC,
    reps=1,
    hw_loop=0,
    rg=18,
    xf_bufs=2,
    xw_bufs=3,
    c32_bufs=4,
    ob_bufs=2,
    dt_bufs=6,
    tr_chunks=2,
    ch_eng="scalar",
    cl_eng="gpsimd",
    split="full",
    layout="planes",
):
    """F(2,3) Winograd + fp8e4 hi/lo-split DoubleRow matmuls.

    Same Winograd structure as build_conv_bass_wino, but each comp is split
    into fp8e4 hi/lo planes (c = ch + cl exactly to ~2^-8 relative) and the
    PE runs one DoubleRow matmul per (comp, ci, kh) contracting both planes
    at 0.5 cycles/row — halving the stream cycles vs bf16.  Transformed
    weights {+-1/2, +-1, +-3/2} are exact in fp8e4; both DR planes carry the
    same weights.  Split work is spread across DVE (comp f32), ACT (hi
    extract), and Pool (lo residual) so no vector engine exceeds PE time.
    """
    import contextlib

    import concourse.mybir as mybir
    import concourse.tile as tile
    from concourse import bacc

    fp32 = mybir.dt.float32
    fp8 = mybir.dt.float8e4

    nc = bacc.Bacc("TRN2", target_bir_lowering=False, debug=False)

    x_in = nc.dram_tensor("x", [npc, _C, _H, _W], fp32, kind="ExternalInput")
    w_in = nc.dram_tensor("w", [_KH * _KW, _C, _CO], fp32, kind="ExternalInput")
    out = nc.dram_tensor("out", [npc, _CO, _HO, _WO], fp32, kind="ExternalOutput")

    n_ci = _C // 128   # 2
    n_co = _CO // 128  # 2
    nk = _KH * _KW     # 9
    WP = _WO // 2      # 27 output pairs
    G = _HO // rg      # row groups
    assert _HO % rg == 0

    ch_engine = {"scalar": nc.scalar, "vector": nc.vector}[ch_eng]
    cl_engine = {"gpsimd": nc.gpsimd, "vector": nc.vector}[cl_eng]

    with tile.TileContext(nc) as tc:
        with (
            tc.tile_pool(name="wstage", bufs=1) as wstage_pool,
            tc.tile_pool(name="wsign", bufs=1) as wsign_pool,
            tc.tile_pool(name="wtmp", bufs=2) as wtmp_pool,
            tc.tile_pool(name="wg", bufs=n_ci) as wg_pool,
            tc.tile_pool(name="xf", bufs=xf_bufs) as xf_pool,
            tc.tile_pool(name="xw", bufs=xw_bufs) as xw_pool,
            tc.tile_pool(name="c32", bufs=c32_bufs) as c32_pool,
            tc.tile_pool(name="dt", bufs=dt_bufs) as dt_pool,
            tc.tile_pool(name="ob", bufs=ob_bufs) as ob_pool,
            tc.tile_pool(name="ps", bufs=8, space="PSUM") as ps_pool,
        ):
            # ---- weight prep: binarize, 1D-transform along kw, duplicate
            # into both DoubleRow planes.  wg8[ci][:, p, kh*4+c, co]:
            # c=0 -> sign0, c=1 -> (s0+s1+s2)/2, c=2 -> (s0-s1+s2)/2,
            # c=3 -> sign2  — all exact in fp8e4.
            wg8 = {}
            for ci_t in range(n_ci):
                stage = wstage_pool.tile([128, nk, _CO], fp32)
                nc.scalar.dma_start(
                    stage[:],
                    w_in[:, ci_t * 128 : (ci_t + 1) * 128, :].rearrange(
                        "k c o -> c k o"
                    ),
                )
                sg = wsign_pool.tile([128, nk, _CO], fp32)
                nc.vector.tensor_scalar(
                    sg[:], stage[:], 0.0, 0.5,
                    mybir.AluOpType.is_ge, mybir.AluOpType.subtract,
                )  # +-0.5 = sign/2
                if layout == "wsplit":
                    # one contiguous [2,128] weight block per (col, co_t)
                    wt = wg_pool.tile([128, _KH * 4, n_co, 2, 128], fp8)
                else:
                    wt = wg_pool.tile([128, 2, _KH * 4, _CO], fp8)

                def wwrite(p, col, src, scale=None):
                    if layout == "wsplit":
                        for co in range(n_co):
                            dst = wt[:, col, co, p, :]
                            sco = src[:, co * 128 : (co + 1) * 128]
                            if scale is None:
                                nc.scalar.copy(dst, sco)
                            else:
                                nc.scalar.mul(dst, sco, scale)
                    else:
                        dst = wt[:, p, col, :]
                        if scale is None:
                            nc.scalar.copy(dst, src)
                        else:
                            nc.scalar.mul(dst, src, scale)
                for kh in range(_KH):
                    s0 = sg[:, kh * _KW + 0, :]
                    s1 = sg[:, kh * _KW + 1, :]
                    s2 = sg[:, kh * _KW + 2, :]
                    t02 = wtmp_pool.tile([128, _CO], fp32, name="t02", tag="t02")
                    nc.vector.tensor_add(t02[:], s0, s2)
                    tp = wtmp_pool.tile([128, _CO], fp32, name="tp", tag="tp")
                    nc.vector.tensor_add(tp[:], t02[:], s1)
                    tm = wtmp_pool.tile([128, _CO], fp32, name="tm", tag="tm")
                    nc.vector.tensor_sub(tm[:], t02[:], s1)
                    for p in range(2):
                        wwrite(p, kh * 4 + 0, s0, 2.0)
                        wwrite(p, kh * 4 + 1, tp[:])
                        wwrite(p, kh * 4 + 2, tm[:])
                        wwrite(p, kh * 4 + 3, s2, 2.0)
                wg8[ci_t] = wt

            loop_cm = (
                tc.For_i(
                    0,
                    hw_loop,
                    1,
                    hint_engines=(
                        mybir.EngineType.PE,
                        mybir.EngineType.Activation,
                        mybir.EngineType.DVE,
                        mybir.EngineType.SP,
                        mybir.EngineType.Pool,
                    ),
                )
                if hw_loop > 0
                else contextlib.nullcontext()
            )
            comp_defs = [
                (0, 2, "subtract"),  # c1 = d0 - d2
                (1, 2, "add"),       # c2 = d1 + d2
                (2, 1, "subtract"),  # c3 = d2 - d1
                (1, 3, "subtract"),  # c4 = d1 - d3
            ]
            with loop_cm:
                for rep in range(reps):
                    for n in range(npc):
                        xw = {}
                        for ci_t in range(n_ci):
                            xf = xf_pool.tile([128, _H * _W], fp32)
                            nc.sync.dma_start(
                                xf[:], x_in[n, ci_t * 128 : (ci_t + 1) * 128, :, :]
                            )
                            v = xf[:].rearrange(
                                "p (h wp t) -> p h wp t", wp=_W // 2, t=2
                            )
                            dpick = {
                                0: v[:, :, 0:WP, 0],
                                1: v[:, :, 0:WP, 1],
                                2: v[:, :, 1 : WP + 1, 0],
                                3: v[:, :, 1 : WP + 1, 1],
                            }
                            if layout != "ilv":
                                xc = xw_pool.tile(
                                    [128, 4, 2, _H, WP], fp8, name="xc", tag="xc"
                                )
                                xsl = lambda c, p, r: xc[:, c, p, r, :]
                                lo_all = lambda: xc[:, :, 1, :, :]
                            else:
                                xc = xw_pool.tile(
                                    [128, 4, _H, WP, 2], fp8, name="xc", tag="xc"
                                )
                                xsl = lambda c, p, r: xc[:, c, r, :, p]
                                lo_all = lambda: xc[:, :, :, :, 1]
                            hstep = -(-_H // tr_chunks)
                            if split == "hi_only":
                                # timing ablation: lo plane zeroed, hi written
                                # directly by the comp op (1 op per chunk)
                                nc.vector.memset(lo_all(), 0.0)
                            for c, (ia, ib, opname) in enumerate(comp_defs):
                                op = getattr(mybir.AluOpType, opname)
                                for h0 in range(0, _H, hstep):
                                    h1 = min(h0 + hstep, _H)
                                    r = slice(h0, h1)
                                    if split == "hi_only":
                                        nc.vector.tensor_tensor(
                                            xsl(c, 0, r),
                                            dpick[ia][:, r], dpick[ib][:, r], op,
                                        )
                                        continue
                                    c32 = c32_pool.tile(
                                        [128, h1 - h0, WP], fp32,
                                        name="c32", tag="c32",
                                    )
                                    nc.vector.tensor_tensor(
                                        c32[:], dpick[ia][:, r], dpick[ib][:, r], op
                                    )
                                    if ch_eng == "vector":
                                        nc.vector.tensor_copy(
                                            xsl(c, 0, r), c32[:]
                                        )
                                    else:
                                        ch_engine.copy(xsl(c, 0, r), c32[:])
                                    cl_engine.tensor_sub(
                                        xsl(c, 1, r), c32[:], xsl(c, 0, r)
                                    )
                            xw[ci_t] = xc

                        for co_t in range(n_co):
                            ob = ob_pool.tile(
                                [128, _HO, _WO], fp32, name="ob", tag="ob"
                            )
                            obv = ob[:].rearrange("p h (wp t) -> p h wp t", t=2)
                            for g in range(G):
                                mq = [
                                    ps_pool.tile(
                                        [128, rg, WP], fp32, name="mq", tag="mq"
                                    )
                                    for _ in range(4)
                                ]
                                for c in range(4):
                                    for ci_t in range(n_ci):
                                        for kh in range(_KH):
                                            co_sl = slice(
                                                co_t * 128, (co_t + 1) * 128
                                            )
                                            rsl = slice(
                                                g * rg + kh, g * rg + kh + rg
                                            )
                                            if layout == "wsplit":
                                                w_ap = wg8[ci_t][
                                                    :, kh * 4 + c, co_t, :, :
                                                ]
                                            else:
                                                w_ap = wg8[ci_t][
                                                    :, :, kh * 4 + c, co_sl
                                                ]
                                            if layout != "ilv":
                                                rhs = xw[ci_t][:, c, :, rsl, :]
                                            else:
                                                rhs = xw[ci_t][
                                                    :, c, rsl, :, :
                                                ].rearrange(
                                                    "p r w t -> p t (r w)"
                                                )
                                            nc.tensor.matmul(
                                                mq[c][:, :, :],
                                                w_ap,
                                                rhs,
                                                start=(ci_t == 0 and kh == 0),
                                                stop=(
                                                    ci_t == n_ci - 1
                                                    and kh == _KH - 1
                                                ),
                                                perf_mode=(
                                                    mybir.MatmulPerfMode.DoubleRow
                                                ),
                                            )
                                rows = slice(g * rg, (g + 1) * rg)
                                s2 = dt_pool.tile([128, rg, WP], fp32, name="s2", tag="s2")
                                nc.scalar.copy(s2[:], mq[1][:])
                                s3 = dt_pool.tile([128, rg, WP], fp32, name="s3", tag="s3")
                                nc.scalar.copy(s3[:], mq[2][:])
                                t0 = dt_pool.tile([128, rg, WP], fp32, name="t0", tag="t0")
                                nc.vector.tensor_add(t0[:], mq[0][:], s2[:])
                                nc.vector.tensor_add(
                                    obv[:, rows, :, 0], t0[:], s3[:]
                                )
                                t1 = dt_pool.tile([128, rg, WP], fp32, name="t1", tag="t1")
                                nc.vector.tensor_sub(t1[:], s2[:], s3[:])
                                nc.vector.tensor_sub(
                                    obv[:, rows, :, 1], t1[:], mq[3][:]
                                )
                                if g == G - 1:
                                    nc.scalar.dma_start(
                                        out[
                                            n,
                                            co_t * 128 : (co_t + 1) * 128,
                                            :,
                                            :,
                                        ],
                                        ob[:],
                                    )

    nc.compile()
    return nc


def build_conv_f43(
    npc=_NPC,
    reps=1,
    hw_loop=0,
    xf_bufs=2,
    xw_bufs=3,
    c32_bufs=4,
    ob_bufs=2,
    dt_bufs=2,
    tr_chunks=1,
    tr_pool=False,
    det_pool=True,
    det_mode="full",
    ps_pairs=True,
    tr_mode="full",
):
    """F(4,3) Winograd along W (kh direct), bf16.

    6 comps per 4 output cols -> 25% fewer PE stream cycles than F(2,3).
    Comps r0..r5 = scaled B^T rows (12 fused DVE/Pool ops per ci-chunk);
    weight cols are exact bf16 integers {+-1/4,..,+-7}; detransform uses
    scaled A^T with /6-family f32 scalars.  PSUM pairs (m1,m3),(m2,m4),
    (m0,m5) live in 2-bank tiles so detransform ops cover 756 elements.
    W padded to 60 on SBUF (memset), out cols 54..55 discarded.
    """
    import contextlib

    import concourse.mybir as mybir
    import concourse.tile as tile
    from concourse import bacc

    fp32 = mybir.dt.float32
    bf16 = mybir.dt.bfloat16

    nc = bacc.Bacc("TRN2", target_bir_lowering=False, debug=False)

    x_in = nc.dram_tensor("x", [npc, _C, _H, _W], fp32, kind="ExternalInput")
    w_in = nc.dram_tensor("w", [_KH * _KW, _C, _CO], fp32, kind="ExternalInput")
    out = nc.dram_tensor("out", [npc, _CO, _HO, _WO], fp32, kind="ExternalOutput")

    n_ci = _C // 128   # 2
    n_co = _CO // 128  # 2
    nk = _KH * _KW     # 9
    WT = 14            # wtiles (56 padded cols / 4)
    WPAD = 60
    rg = 27            # out rows per group
    G = _HO // rg      # 2
    NF = rg * WT       # 378 matmul free size

    AL = mybir.AluOpType
    sub, add, mult = AL.subtract, AL.add, AL.mult

    with tile.TileContext(nc) as tc:
        with (
            tc.tile_pool(name="wstage", bufs=1) as wstage_pool,
            tc.tile_pool(name="wsign", bufs=1) as wsign_pool,
            tc.tile_pool(name="wtmp", bufs=2) as wtmp_pool,
            tc.tile_pool(name="wg", bufs=n_ci) as wg_pool,
            tc.tile_pool(name="xf", bufs=xf_bufs) as xf_pool,
            tc.tile_pool(name="xw", bufs=xw_bufs) as xw_pool,
            tc.tile_pool(name="c32", bufs=c32_bufs) as c32_pool,
            tc.tile_pool(name="dt", bufs=dt_bufs) as dt_pool,
            tc.tile_pool(name="ob", bufs=ob_bufs) as ob_pool,
            tc.tile_pool(
                name="ps",
                bufs=(4 if ps_pairs is True else 8),
                space="PSUM",
            ) as ps_pool,
        ):
            # ---- weight prep: 6 transformed cols per kh, exact bf16.
            # From sg = sign/2: col0 = sg0/2 (= w0/4), col1 = -2*(sg0+sg1+sg2)
            # (= -(w0+w1+w2)), col2 = -2*(sg0-sg1+sg2), col3 = 2sg0+4sg1+8sg2
            # (= w0+2w1+4w2), col4 = 2sg0-4sg1+8sg2, col5 = 2*sg2 (= w2).
            wg8 = {}
            for ci_t in range(n_ci):
                stage = wstage_pool.tile([128, nk, _CO], fp32)
                nc.scalar.dma_start(
                    stage[:],
                    w_in[:, ci_t * 128 : (ci_t + 1) * 128, :].rearrange(
                        "k c o -> c k o"
                    ),
                )
                sg = wsign_pool.tile([128, nk, _CO], fp32)
                nc.vector.tensor_scalar(
                    sg[:], stage[:], 0.0, 0.5, AL.is_ge, sub,
                )  # +-0.5 = sign/2
                wt = wg_pool.tile([128, _KH * 6, _CO], bf16)
                for kh in range(_KH):
                    s0 = sg[:, kh * _KW + 0, :]
                    s1 = sg[:, kh * _KW + 1, :]
                    s2 = sg[:, kh * _KW + 2, :]
                    t02 = wtmp_pool.tile([128, _CO], fp32, name="t02", tag="t02")
                    nc.vector.tensor_add(t02[:], s0, s2)
                    tp = wtmp_pool.tile([128, _CO], fp32, name="tp", tag="tp")
                    nc.vector.tensor_add(tp[:], t02[:], s1)
                    tm = wtmp_pool.tile([128, _CO], fp32, name="tm", tag="tm")
                    nc.vector.tensor_sub(tm[:], t02[:], s1)
                    t3 = wtmp_pool.tile([128, _CO], fp32, name="t3", tag="t3")
                    nc.vector.scalar_tensor_tensor(
                        t3[:], s1, 2.0, s0, mult, add
                    )  # 2sg1+sg0
                    t3b = wtmp_pool.tile([128, _CO], fp32, name="t3b", tag="t3b")
                    nc.vector.scalar_tensor_tensor(
                        t3b[:], s2, 4.0, t3[:], mult, add
                    )  # 4sg2+2sg1+sg0
                    t4 = wtmp_pool.tile([128, _CO], fp32, name="t4", tag="t4")
                    nc.vector.scalar_tensor_tensor(
                        t4[:], s1, -2.0, s0, mult, add
                    )
                    t4b = wtmp_pool.tile([128, _CO], fp32, name="t4b", tag="t4b")
                    nc.vector.scalar_tensor_tensor(
                        t4b[:], s2, 4.0, t4[:], mult, add
                    )
                    base = kh * 6
                    nc.scalar.mul(wt[:, base + 0, :], s0, 0.5)
                    nc.scalar.mul(wt[:, base + 1, :], tp[:], -2.0)
                    nc.scalar.mul(wt[:, base + 2, :], tm[:], -2.0)
                    nc.scalar.mul(wt[:, base + 3, :], t3b[:], 2.0)
                    nc.scalar.mul(wt[:, base + 4, :], t4b[:], 2.0)
                    nc.scalar.mul(wt[:, base + 5, :], s2, 2.0)
                wg8[ci_t] = wt

            loop_cm = (
                tc.For_i(
                    0,
                    hw_loop,
                    1,
                    hint_engines=(
                        mybir.EngineType.PE,
                        mybir.EngineType.Activation,
                        mybir.EngineType.DVE,
                        mybir.EngineType.SP,
                        mybir.EngineType.Pool,
                    ),
                )
                if hw_loop > 0
                else contextlib.nullcontext()
            )
            # psum pair assignment: pair tile [128, 2, 512] (2 banks);
            # comps -> (pair, half): m1->(A,0) m3->(A,1) m2->(B,0) m4->(B,1)
            # m0->(C,0) m5->(C,1)
            pair_of = {1: (0, 0), 3: (0, 1), 2: (1, 0), 4: (1, 1),
                       0: (2, 0), 5: (2, 1)}
            with loop_cm:
                for rep in range(reps):
                    for n in range(npc):
                        xw = {}
                        for ci_t in range(n_ci):
                            xf = None
                            if tr_mode != "none":
                                xf = xf_pool.tile([128, _H, WPAD], fp32)
                                nc.sync.dma_start(
                                    xf[:, :, 0:_W],
                                    x_in[
                                        n, ci_t * 128 : (ci_t + 1) * 128, :, :
                                    ],
                                )
                                nc.vector.memset(xf[:, :, _W:WPAD], 0.0)
                            xv = (
                                xf[:].rearrange(
                                    "p h (wt f) -> p h wt f", f=4
                                )
                                if xf is not None
                                else None
                            )
                            d = None if xv is None else {
                                0: xv[:, :, 0:WT, 0],
                                1: xv[:, :, 0:WT, 1],
                                2: xv[:, :, 0:WT, 2],
                                3: xv[:, :, 0:WT, 3],
                                4: xv[:, :, 1 : WT + 1, 0],
                                5: xv[:, :, 1 : WT + 1, 1],
                            }
                            xc = xw_pool.tile(
                                [128, 6, _H, WT], bf16, name="xc", tag="xc"
                            )
                            eng2 = nc.gpsimd if tr_pool else nc.vector
                            if tr_mode in ("min", "none"):
                                nc.vector.memset(xc[:], 0.25)
                            hstep = -(-_H // tr_chunks)
                            for h0 in (
                                range(0, _H, hstep)
                                if tr_mode == "full"
                                else []
                            ):
                                h1 = min(h0 + hstep, _H)
                                r = slice(h0, h1)
                                hn = h1 - h0

                                def ctile(nm):
                                    return c32_pool.tile(
                                        [128, hn, WT], fp32,
                                        name=nm, tag="c32",
                                    )

                                # DVE: u,v -> r1,r2 ; a0 -> r0
                                u = ctile("u")
                                nc.vector.scalar_tensor_tensor(
                                    u[:], d[2][:, r], -4.0, d[4][:, r], mult, add
                                )
                                v = ctile("v")
                                nc.vector.scalar_tensor_tensor(
                                    v[:], d[1][:, r], -4.0, d[3][:, r], mult, add
                                )
                                eng2.tensor_add(
                                    xc[:, 1, r, :], u[:], v[:]
                                )
                                eng2.tensor_sub(
                                    xc[:, 2, r, :], u[:], v[:]
                                )
                                a0 = ctile("a0")
                                nc.vector.scalar_tensor_tensor(
                                    a0[:], d[2][:, r], -1.25, d[0][:, r],
                                    mult, add,
                                )
                                nc.vector.scalar_tensor_tensor(
                                    xc[:, 0, r, :], a0[:], 4.0, d[4][:, r],
                                    mult, add,
                                )
                                # Pool (or DVE): p,q -> r3,r4 ; a5 -> r5
                                p = ctile("p")
                                eng2.tensor_sub(p[:], d[4][:, r], d[2][:, r])
                                q = ctile("q")
                                eng2.tensor_sub(q[:], d[3][:, r], d[1][:, r])
                                nc.vector.scalar_tensor_tensor(
                                    xc[:, 3, r, :], q[:], 2.0, p[:], mult, add
                                )
                                nc.vector.scalar_tensor_tensor(
                                    xc[:, 4, r, :], q[:], -2.0, p[:], mult, add
                                )
                                a5 = ctile("a5")
                                nc.vector.scalar_tensor_tensor(
                                    a5[:], d[3][:, r], -1.25, d[1][:, r],
                                    mult, add,
                                )
                                nc.vector.scalar_tensor_tensor(
                                    xc[:, 5, r, :], a5[:], 4.0, d[5][:, r],
                                    mult, add,
                                )
                            xw[ci_t] = xc

                        for co_t in range(n_co):
                            co_sl = slice(co_t * 128, (co_t + 1) * 128)
                            ob = ob_pool.tile(
                                [128, _HO, WT * 4], fp32, name="ob", tag="ob"
                            )
                            obv = ob[:].rearrange(
                                "p h (wt f) -> p h wt f", f=4
                            )
                            for g in range(G):
                                if ps_pairs is True:
                                    pairs = [
                                        ps_pool.tile(
                                            [128, 2, 512], fp32,
                                            name="pp", tag="pp",
                                        )
                                        for _ in range(3)
                                    ]
                                    psl = lambda pi, half: pairs[pi][
                                        :, half, 0:NF
                                    ]
                                elif ps_pairs == "exact3d":
                                    sing = [
                                        ps_pool.tile(
                                            [128, rg, WT], fp32,
                                            name="sg6", tag="sg6",
                                        )
                                        for _ in range(6)
                                    ]
                                    psl = lambda pi, half: sing[
                                        pi * 2 + half
                                    ][:].rearrange("p r w -> p (r w)")
                                    psl3 = lambda pi, half: sing[
                                        pi * 2 + half
                                    ][:, :, :]
                                else:
                                    sing = [
                                        ps_pool.tile(
                                            [128, 512], fp32,
                                            name="sg6", tag="sg6",
                                        )
                                        for _ in range(6)
                                    ]
                                    psl = lambda pi, half: sing[
                                        pi * 2 + half
                                    ][:, 0:NF]
                                # matmuls: pairs A,B first (det PR/QS can
                                # start while C computes)
                                for c in (1, 3, 2, 4, 0, 5):
                                    pi, half = pair_of[c]
                                    for ci_t in range(n_ci):
                                        for kh in range(_KH):
                                            nc.tensor.matmul(
                                                psl3(pi, half)
                                                if ps_pairs == "exact3d"
                                                else psl(pi, half),
                                                wg8[ci_t][
                                                    :, kh * 6 + c, co_sl
                                                ],
                                                xw[ci_t][
                                                    :, c,
                                                    g * rg + kh
                                                    : g * rg + kh + rg,
                                                    :,
                                                ],
                                                start=(ci_t == 0 and kh == 0),
                                                stop=(
                                                    ci_t == n_ci - 1
                                                    and kh == _KH - 1
                                                ),
                                            )
                                # detransform.  Phase 1 releases PSUM
                                # in 4 ops (ACT stage + 3 DVE); phase 2 is
                                # SBUF-only on ACT (scalar muls) + Pool
                                # (plain adds).
                                dp = nc.gpsimd if det_pool else nc.vector
                                rows = slice(g * rg, (g + 1) * rg)
                                if ps_pairs is True:
                                    pA = pairs[0][:, :, 0:NF]
                                    pB = pairs[1][:, :, 0:NF]
                                    pC = pairs[2][:, :, 0:NF]
                                if det_mode in ("min", "minc"):
                                    for pi in range(4):
                                        src = psl(pi % 3, pi // 3)
                                        if det_mode == "minc":
                                            dst = ob[
                                                :, rows, pi * WT : (pi + 1) * WT
                                            ]
                                        else:
                                            dst = obv[:, rows, :, pi]
                                        nc.scalar.copy(
                                            dst,
                                            src.rearrange(
                                                "p (r w) -> p r w", w=WT
                                            ),
                                        )
                                    continue
                                sB = dt_pool.tile(
                                    [128, 2, NF], fp32, name="sB", tag="sB"
                                )
                                nc.scalar.copy(sB[:], pB)
                                PR = dt_pool.tile(
                                    [128, 2, NF], fp32, name="PR", tag="PR"
                                )
                                nc.vector.tensor_add(PR[:], pA, sB[:])
                                QS = dt_pool.tile(
                                    [128, 2, NF], fp32, name="QS", tag="QS"
                                )
                                nc.vector.tensor_sub(QS[:], pA, sB[:])
                                C05 = dt_pool.tile(
                                    [128, 2, rg, WT], fp32, name="C05", tag="C05"
                                )
                                nc.vector.tensor_copy(
                                    C05[:],
                                    pC.rearrange("p t (r w) -> p t r w", w=WT),
                                )
                                # phase 2: ACT scalar muls
                                PRm = dt_pool.tile(
                                    [128, 2, rg, WT], fp32, name="PRm", tag="PRm"
                                )
                                nc.scalar.mul(
                                    PRm[:],
                                    PR[:].rearrange(
                                        "p t (r w) -> p t r w", w=WT
                                    ),
                                    1.0 / 6.0,
                                )  # [P/6, R/6]
                                R24 = dt_pool.tile(
                                    [128, rg, WT], fp32, name="R24", tag="R24"
                                )
                                nc.scalar.mul(
                                    R24[:],
                                    PR[:, 1, :].rearrange(
                                        "p (r w) -> p r w", w=WT
                                    ),
                                    1.0 / 24.0,
                                )
                                QSm = dt_pool.tile(
                                    [128, 2, rg, WT], fp32, name="QSm", tag="QSm"
                                )
                                nc.scalar.mul(
                                    QSm[:],
                                    QS[:].rearrange(
                                        "p t (r w) -> p t r w", w=WT
                                    ),
                                    1.0 / 6.0,
                                )  # [Q/6, S/6]
                                S12 = dt_pool.tile(
                                    [128, rg, WT], fp32, name="S12", tag="S12"
                                )
                                nc.scalar.mul(
                                    S12[:],
                                    QS[:, 1, :].rearrange(
                                        "p (r w) -> p r w", w=WT
                                    ),
                                    1.0 / 12.0,
                                )
                                S3 = dt_pool.tile(
                                    [128, rg, WT], fp32, name="S3", tag="S3"
                                )
                                nc.scalar.mul(
                                    S3[:],
                                    QS[:, 1, :].rearrange(
                                        "p (r w) -> p r w", w=WT
                                    ),
                                    1.0 / 3.0,
                                )
                                # phase 3: plain adds (Pool), SBUF only
                                dp.tensor_add(
                                    obv[:, rows, :, 2], PRm[:, 0], PRm[:, 1]
                                )  # y2
                                dp.tensor_add(
                                    obv[:, rows, :, 1], QSm[:, 0], S12[:]
                                )  # y1
                                av = dt_pool.tile(
                                    [128, rg, WT], fp32, name="av", tag="av"
                                )
                                dp.tensor_add(av[:], PRm[:, 0], C05[:, 0])
                                dp.tensor_add(
                                    obv[:, rows, :, 0], av[:], R24[:]
                                )  # y0
                                tv = dt_pool.tile(
                                    [128, rg, WT], fp32, name="tv", tag="tv"
                                )
                                dp.tensor_add(tv[:], QSm[:, 0], S3[:])
                                dp.tensor_add(
                                    obv[:, rows, :, 3], tv[:], C05[:, 1]
                                )  # y3
                            nc.scalar.dma_start(
                                out[n, co_sl, :, :], ob[:, :, 0:_WO]
                            )

    nc.compile()
    return nc


_CACHED_NC = None


def _host_weight_layout(W):
    # OIHW [co,ci,kh,kw] -> [kh*kw, ci, co]; layout only, no arithmetic.
    return np.ascontiguousarray(
        np.transpose(np.asarray(W, dtype=np.float32), (2, 3, 1, 0)).reshape(
            _KH * _KW, _C, _CO
        )
    )


def _in_maps(inputs, n_cores=_NCORES, npc=_NPC):
    X = np.asarray(inputs["X"], dtype=np.float32)
    Wt = _host_weight_layout(inputs["W"])
    return [{"x": X[c * npc : (c + 1) * npc], "w": Wt} for c in range(n_cores)]


def _spot_check(X, W, out):
    """Tiny host-side sanity sample (a few output columns) to catch a
    transiently-wedged device run; the conv itself always runs on device."""
    ws = np.where(np.asarray(W) >= 0, 1.0, -1.0).astype(np.float32)
    X = np.asarray(X, dtype=np.float32)
    worst = 0.0
    for n in (0, _N // 2, _N - 1):
        for (h, w) in ((0, 0), (27, 27), (53, 53)):
            patch = X[n, :, h : h + 3, w : w + 3]
            exp = np.einsum("ckl,ockl->o", patch, ws)
            got = out[n, :, h, w]
            rel = np.abs(got - exp).max() / max(np.abs(exp).max(), 1.0)
            worst = max(worst, float(rel))
    return worst


def kernel(X, W):
    import os

    # NTFF tracing is unavailable under this axon image (antenv.axon_hooks
    # missing); make sure a stray BASS_TRACE can't route us into it.
    os.environ["BASS_NEVER_TRACE"] = "1"
    from concourse.bass_utils import run_bass_kernel_spmd

    global _CACHED_NC
    if _CACHED_NC is None:
        _CACHED_NC = build_conv_bass_wino(_NPC)
    nc = _CACHED_NC

    in_maps = _in_maps({"X": X, "W": W})
    out = None
    for attempt in range(3):
        res = run_bass_kernel_spmd(nc, in_maps, core_ids=list(range(_NCORES)))
        out = np.concatenate(
            [res.results[c]["out"] for c in range(_NCORES)], axis=0
        )
        if _spot_check(X, W, out) < 1e-2:
            break
    return out

